# revision 1
# baseline (speedup 1.0000x reference)
import numpy as np
from contextlib import ExitStack

import concourse.bass as bass
import concourse.bacc as bacc
import concourse.mybir as mybir
from concourse.tile import TileContext
from concourse.bass_utils import run_bass_kernel_spmd

B, T, K, D = 512, 2048, 8, 32
DT = 0.05
NCORES = 8
BL = B // NCORES          # 64 paths per core
TC = 128                  # timesteps per chunk
NCH = T // TC
SG = 16                   # diff matmul steps per PSUM bank fill

F32 = mybir.dt.float32
F32R = mybir.dt.float32r

_cache = {}


def _build():
    nc = bacc.Bacc()
    z0 = nc.declare_dram_parameter("z0", [BL, D], F32, isOutput=False)
    sp = nc.declare_dram_parameter("sp", [T, BL, K], F32, isOutput=False)
    nz = nc.declare_dram_parameter("nz", [T, BL, D], F32, isOutput=False)
    Rm = nc.declare_dram_parameter("Rm", [D + 1, D * K], F32, isOutput=False)
    Qt = nc.declare_dram_parameter("Qt", [K, D], F32, isOutput=False)
    ys = nc.declare_dram_parameter("ys", [T, BL, D], F32, isOutput=True)

    ctx = ExitStack()
    with TileContext(nc) as tc:
        with (
            tc.tile_pool(name="const", bufs=1) as constp,
            tc.tile_pool(name="io", bufs=2) as iop,
            tc.tile_pool(name="work", bufs=2) as workp,
            tc.tile_pool(name="state", bufs=1) as statep,
            tc.tile_pool(name="ps", bufs=2, space="PSUM") as psp,
            tc.tile_pool(name="psd", bufs=2, space="PSUM") as psdp,
        ):
            # constants
            R_st = constp.tile([D + 1, D * K], F32, tag="Rst")
            nc.sync.dma_start(R_st[:], Rm[:])
            R_sb = constp.tile([D + 1, D * K], F32R, tag="R")
            nc.vector.tensor_copy(R_sb[:], R_st[:])
            Qt_sb = constp.tile([K, D], F32, tag="Qt")
            nc.sync.dma_start(Qt_sb[:], Qt[:])
            z0_sb = constp.tile([BL, D], F32, tag="z0")
            nc.sync.dma_start(z0_sb[:], z0[:])

            # transposed state (aug with ones row), persistent
            zT = statep.tile([D + 1, BL], F32R, tag="zT")
            ones = constp.tile([1, BL], F32, tag="ones")
            nc.vector.memset(ones[:], 1.0)
            nc.vector.tensor_copy(zT[D : D + 1, :], ones[:])

            prev = z0_sb[:]  # [BL, D] AP holding z_{t-1}

            for c in range(NCH):
                t0 = c * TC
                # ---- chunk DMAs ----
                sp_ch = iop.tile([BL, TC, K], F32, tag="sp")
                nc.sync.dma_start(
                    sp_ch[:], sp[t0 : t0 + TC].rearrange("t b k -> b t k")
                )
                nz_ch = iop.tile([BL, TC, D], F32, tag="nz")
                nc.sync.dma_start(
                    nz_ch[:], nz[t0 : t0 + TC].rearrange("t b d -> b t d")
                )
                wT_ch = iop.tile([K, TC, BL], F32, tag="wT")
                nc.sync.dma_start(
                    wT_ch[:], sp[t0 : t0 + TC].rearrange("t b k -> k t b")
                )

                # ---- bulk prep ----
                wsum = workp.tile([BL, TC], F32, tag="wsum")
                nc.vector.tensor_reduce(
                    wsum[:], sp_ch[:], mybir.AxisListType.X, mybir.AluOpType.add
                )
                recip = workp.tile([BL, TC], F32, tag="recip")
                nc.vector.reciprocal(recip[:], wsum[:])
                recdt = workp.tile([BL, TC], F32, tag="recdt")
                nc.vector.tensor_scalar_mul(recdt[:], recip[:], DT)
                wn = workp.tile([BL, TC, K], F32, tag="wn")
                nc.vector.tensor_mul(
                    wn[:], sp_ch[:], recdt[:].unsqueeze(2).broadcast_to((BL, TC, K))
                )

                # diffusion magnitudes via PE: diffE[b, t, i] = sum_k w[b,t,k] Qt[k,i]
                dfn = workp.tile([BL, TC, D], F32, tag="dfn")
                for g in range(TC // SG):
                    psd = psdp.tile([BL, SG * D], F32, tag="psd")
                    for s in range(SG):
                        tt = g * SG + s
                        nc.tensor.matmul(
                            psd[:, s * D : (s + 1) * D],
                            wT_ch[:, tt, :],
                            Qt_sb[:],
                            start=True,
                            stop=True,
                        )
                    nc.scalar.copy(
                        dfn[:, g * SG : (g + 1) * SG, :].rearrange("b t d -> b (t d)"),
                        psd[:],
                    )
                # dfn *= noise ; dfn *= 1/wsum
                nc.vector.tensor_mul(dfn[:], dfn[:], nz_ch[:])
                nc.vector.tensor_mul(
                    dfn[:], dfn[:], recip[:].unsqueeze(2).broadcast_to((BL, TC, D))
                )

                ys_st = iop.tile([BL, TC, D], F32, tag="ys")

                # ---- serial scan over the chunk ----
                for s in range(TC):
                    zTf = workp.tile([D, BL], F32, tag="zTf")
                    nc.vector.transpose(zTf[:, 0:32], prev[0:32, :])
                    nc.vector.transpose(zTf[:, 32:64], prev[32:64, :])
                    nc.vector.tensor_copy(zT[0:D, :], zTf[:])
                    Y = psp.tile([BL, D * K], F32, tag="Y")
                    nc.tensor.matmul(
                        Y[:], zT[:], R_sb[:], start=True, stop=True
                    )
                    P = workp.tile([BL, D, K], F32, tag="P")
                    nc.vector.tensor_mul(
                        P[:],
                        Y[:].rearrange("b (d k) -> b d k", k=K),
                        wn[:, s, :].unsqueeze(1).broadcast_to((BL, D, K)),
                    )
                    u0 = workp.tile([BL, D], F32, tag="u0")
                    nc.vector.tensor_reduce(
                        u0[:], P[:], mybir.AxisListType.X, mybir.AluOpType.add
                    )
                    tu = workp.tile([BL, D], F32, tag="tu")
                    nc.vector.tensor_add(tu[:], u0[:], dfn[:, s, :])
                    nc.vector.tensor_add(ys_st[:, s, :], tu[:], prev)
                    prev = ys_st[:, s, :]

                nc.sync.dma_start(
                    ys[t0 : t0 + TC].rearrange("t b d -> b t d"), ys_st[:]
                )
    ctx.close()
    nc.finalize()
    return nc


def kernel(z0, s_probs, noise, A_s, b_s, Q_chol):
    if "nc" not in _cache:
        _cache["nc"] = _build()
    nc = _cache["nc"]

    A_s = np.asarray(A_s, np.float32)
    b_s = np.asarray(b_s, np.float32)
    Q_chol = np.asarray(Q_chol, np.float32)
    z0 = np.asarray(z0, np.float32)
    s_probs = np.ascontiguousarray(np.asarray(s_probs, np.float32))
    noise = np.ascontiguousarray(np.asarray(noise, np.float32))

    # R[j, i*K+k] = A[k,i,j] ; R[D, i*K+k] = b_s[k,i]
    Ahat = A_s
    Rm = np.empty((D + 1, D * K), np.float32)
    Rm[:D, :] = Ahat.transpose(2, 1, 0).reshape(D, D * K)
    Rm[D, :] = b_s.T.reshape(D * K)
    Qt = (Q_chol * np.float32(np.sqrt(DT))).astype(np.float32)

    in_maps = []
    for c in range(NCORES):
        b0 = c * BL
        in_maps.append(
            {
                "z0": np.ascontiguousarray(z0[b0 : b0 + BL]),
                "sp": np.ascontiguousarray(s_probs[:, b0 : b0 + BL, :]),
                "nz": np.ascontiguousarray(noise[:, b0 : b0 + BL, :]),
                "Rm": Rm,
                "Qt": Qt,
            }
        )

    res = run_bass_kernel_spmd(nc, in_maps, list(range(NCORES))).results
    out = np.empty((T, B, D), np.float32)
    for c in range(NCORES):
        out[:, c * BL : (c + 1) * BL, :] = res[c]["ys"]
    return out



# revision 2
# speedup vs baseline: 3.4856x; 3.4856x over previous
import numpy as np
from contextlib import ExitStack

import concourse.bass as bass
import concourse.bacc as bacc
import concourse.mybir as mybir
from concourse.tile import TileContext

B, T, K, D = 512, 2048, 8, 32
DT = 0.05
NCORES = 8
BL = B // NCORES          # 64 paths per core
TC = 128                  # timesteps per chunk
NCH = T // TC
SG = 16                   # diff matmul steps per PSUM bank fill

F32 = mybir.dt.float32
F32R = mybir.dt.float32r
BF16 = mybir.dt.bfloat16

_cache = {}


def _build():
    nc = bacc.Bacc()
    z0 = nc.declare_dram_parameter("z0", [BL, D], F32, isOutput=False)
    sp = nc.declare_dram_parameter("sp", [T, BL, K], BF16, isOutput=False)
    nz = nc.declare_dram_parameter("nz", [T, BL, D], BF16, isOutput=False)
    Rm = nc.declare_dram_parameter("Rm", [D + 1, D * K], F32, isOutput=False)
    Qt = nc.declare_dram_parameter("Qt", [K, D], BF16, isOutput=False)
    ys = nc.declare_dram_parameter("ys", [T, BL, D], BF16, isOutput=True)

    ctx = ExitStack()
    with TileContext(nc) as tc:
        with (
            tc.tile_pool(name="const", bufs=1) as constp,
            tc.tile_pool(name="io", bufs=2) as iop,
            tc.tile_pool(name="work", bufs=2) as workp,
            tc.tile_pool(name="state", bufs=1) as statep,
            tc.tile_pool(name="ps", bufs=2, space="PSUM") as psp,
            tc.tile_pool(name="psd", bufs=2, space="PSUM") as psdp,
        ):
            # constants
            R_st = constp.tile([D + 1, D * K], F32, tag="Rst")
            nc.sync.dma_start(R_st[:], Rm[:])
            R_sb = constp.tile([D + 1, D * K], F32R, tag="R")
            nc.vector.tensor_copy(R_sb[:], R_st[:])
            Qt_sb = constp.tile([K, D], BF16, tag="Qt")
            nc.sync.dma_start(Qt_sb[:], Qt[:])
            z0_sb = constp.tile([BL, D], F32, tag="z0")
            nc.sync.dma_start(z0_sb[:], z0[:])

            # transposed state (aug with ones row), persistent
            zT = statep.tile([D + 1, BL], F32R, tag="zT")
            ones = constp.tile([1, BL], F32, tag="ones")
            nc.vector.memset(ones[:], 1.0)
            nc.vector.tensor_copy(zT[D : D + 1, :], ones[:])

            prev = z0_sb[:]  # [BL, D] AP holding z_{t-1}

            for c in range(NCH):
                t0 = c * TC
                # ---- chunk DMAs ----
                sp_ch = iop.tile([BL, TC, K], BF16, tag="sp")
                nc.sync.dma_start(
                    sp_ch[:], sp[t0 : t0 + TC].rearrange("t b k -> b t k")
                )
                nz_ch = iop.tile([BL, TC, D], BF16, tag="nz")
                nc.sync.dma_start(
                    nz_ch[:], nz[t0 : t0 + TC].rearrange("t b d -> b t d")
                )
                wT_ch = iop.tile([K, TC, BL], BF16, tag="wT")
                nc.sync.dma_start(
                    wT_ch[:], sp[t0 : t0 + TC].rearrange("t b k -> k t b")
                )

                # ---- bulk prep ----
                wsum = workp.tile([BL, TC], F32, tag="wsum")
                nc.vector.tensor_reduce(
                    wsum[:], sp_ch[:], mybir.AxisListType.X, mybir.AluOpType.add
                )
                recip = workp.tile([BL, TC], F32, tag="recip")
                nc.vector.reciprocal(recip[:], wsum[:])
                recdt = workp.tile([BL, TC], F32, tag="recdt")
                nc.vector.tensor_scalar_mul(recdt[:], recip[:], DT)
                wn = workp.tile([BL, TC, K], F32, tag="wn")
                nc.vector.tensor_mul(
                    wn[:], sp_ch[:], recdt[:].unsqueeze(2).broadcast_to((BL, TC, K))
                )

                # diffusion magnitudes via PE: diffE[b, t, i] = sum_k w[b,t,k] Qt[k,i]
                dfn = workp.tile([BL, TC, D], F32, tag="dfn")
                for g in range(TC // SG):
                    psd = psdp.tile([BL, SG * D], F32, tag="psd")
                    for s in range(SG):
                        tt = g * SG + s
                        nc.tensor.matmul(
                            psd[:, s * D : (s + 1) * D],
                            wT_ch[:, tt, :],
                            Qt_sb[:],
                            start=True,
                            stop=True,
                        )
                    nc.scalar.copy(
                        dfn[:, g * SG : (g + 1) * SG, :].rearrange("b t d -> b (t d)"),
                        psd[:],
                    )
                # dfn *= noise ; dfn *= 1/wsum
                nc.vector.tensor_mul(dfn[:], dfn[:], nz_ch[:])
                nc.vector.tensor_mul(
                    dfn[:], dfn[:], recip[:].unsqueeze(2).broadcast_to((BL, TC, D))
                )

                ys_st = iop.tile([BL, TC, D], F32, tag="ys")
                ys_bf = iop.tile([BL, TC, D], BF16, tag="ysb")

                # ---- serial scan over the chunk ----
                for s in range(TC):
                    zTf = workp.tile([D, BL], F32, tag="zTf")
                    nc.vector.transpose(zTf[:, 0:32], prev[0:32, :])
                    nc.vector.transpose(zTf[:, 32:64], prev[32:64, :])
                    nc.vector.tensor_copy(zT[0:D, :], zTf[:])
                    Y = psp.tile([BL, D * K], F32, tag="Y")
                    nc.tensor.matmul(
                        Y[:], zT[:], R_sb[:], start=True, stop=True
                    )
                    P = workp.tile([BL, D, K], F32, tag="P")
                    nc.vector.tensor_mul(
                        P[:],
                        Y[:].rearrange("b (d k) -> b d k", k=K),
                        wn[:, s, :].unsqueeze(1).broadcast_to((BL, D, K)),
                    )
                    u0 = workp.tile([BL, D], F32, tag="u0")
                    nc.vector.tensor_reduce(
                        u0[:], P[:], mybir.AxisListType.X, mybir.AluOpType.add
                    )
                    tu = workp.tile([BL, D], F32, tag="tu")
                    nc.vector.tensor_add(tu[:], u0[:], dfn[:, s, :])
                    nc.vector.tensor_add(ys_st[:, s, :], tu[:], prev)
                    prev = ys_st[:, s, :]
                    nc.scalar.copy(ys_bf[:, s, :], ys_st[:, s, :])

                nc.sync.dma_start(
                    ys[t0 : t0 + TC].rearrange("t b d -> b t d"), ys_bf[:]
                )
    ctx.close()
    nc.finalize()
    return nc


def _host_params(A_s, b_s, Q_chol):
    A_s = np.asarray(A_s, np.float32)
    b_s = np.asarray(b_s, np.float32)
    Q_chol = np.asarray(Q_chol, np.float32)
    Rm = np.empty((D + 1, D * K), np.float32)
    Rm[:D, :] = A_s.transpose(2, 1, 0).reshape(D, D * K)
    Rm[D, :] = b_s.T.reshape(D * K)
    Qt = (Q_chol * np.float32(np.sqrt(DT))).astype(np.float32)
    return Rm, Qt


def _get_runtime():
    if "fn" in _cache:
        return _cache
    import jax
    import jax.numpy as jnp
    from jax.sharding import Mesh, PartitionSpec as P, NamedSharding
    from jax.experimental.shard_map import shard_map
    from concourse.bass2jax import (
        _bass_exec_p,
        install_neuronx_cc_hook,
        partition_id_tensor,
    )

    nc = _build()
    install_neuronx_cc_hook()

    in_names, out_names, out_avals = [], [], []
    for alloc in nc.m.functions[0].allocations:
        if not isinstance(alloc, mybir.MemoryLocationSet):
            continue
        name = alloc.memorylocations[0].name
        if alloc.kind == "ExternalInput":
            if nc.partition_id_tensor is None or name != nc.partition_id_tensor.name:
                in_names.append(name)
        elif alloc.kind == "ExternalOutput":
            out_names.append(name)
            out_avals.append(
                jax.core.ShapedArray(tuple(alloc.tensor_shape), mybir.dt.np(alloc.dtype))
            )
    n_params = len(in_names)
    all_names = in_names + out_names
    if nc.partition_id_tensor is not None:
        all_names = all_names + [nc.partition_id_tensor.name]

    def _body(*args):
        operands = list(args)
        if nc.partition_id_tensor is not None:
            operands.append(partition_id_tensor())
        return tuple(
            _bass_exec_p.bind(
                *operands,
                out_avals=tuple(out_avals),
                in_names=tuple(all_names),
                out_names=tuple(out_names),
                lowering_input_output_aliases=(),
                sim_require_finite=True,
                sim_require_nnan=True,
                nc=nc,
            )
        )

    devices = jax.devices()[:NCORES]
    mesh = Mesh(np.asarray(devices), ("core",))
    spec_map = {
        "z0": P("core", None),
        "sp": P(None, "core", None),
        "nz": P(None, "core", None),
        "Rm": P(None, None),
        "Qt": P(None, None),
    }
    out_spec = P(None, "core", None)
    in_specs = tuple(spec_map[n] for n in in_names) + (out_spec,)
    fn = jax.jit(
        shard_map(
            _body, mesh=mesh, in_specs=in_specs, out_specs=(out_spec,), check_rep=False
        ),
        donate_argnums=(n_params,),
        keep_unused=True,
    )
    bf16 = jnp.bfloat16
    zeros_fn = jax.jit(
        lambda: jnp.zeros((T, B, D), bf16),
        out_shardings=NamedSharding(mesh, out_spec),
    )
    _cache.update(
        fn=fn,
        zeros_fn=zeros_fn,
        in_names=in_names,
        shardings={n: NamedSharding(mesh, spec_map[n]) for n in in_names},
        bf16=bf16,
        device_put=jax.device_put,
    )
    return _cache


def kernel(z0, s_probs, noise, A_s, b_s, Q_chol):
    rt = _get_runtime()
    bf16 = rt["bf16"]
    Rm, Qt = _host_params(A_s, b_s, Q_chol)
    full = {
        "z0": np.asarray(z0, np.float32),
        "sp": np.asarray(s_probs, np.float32).astype(bf16),
        "nz": np.asarray(noise, np.float32).astype(bf16),
        "Rm": Rm,
        "Qt": Qt.astype(bf16),
    }
    zeros = rt["zeros_fn"]()
    dev_in = [rt["device_put"](full[n], rt["shardings"][n]) for n in rt["in_names"]]
    out = rt["fn"](*dev_in, zeros)[0]
    return np.asarray(out).astype(np.float32)
